# revision 7
# baseline (speedup 1.0000x reference)
"""GCN layer (GCNConv + PReLU) on TRN2, SPMD across 8 NeuronCores.

Problem: out = PReLU(A_hat @ (x @ W) + b), A_hat = D^-1/2 (A+I) D^-1/2,
x: [100000, 128] f32, edge_index: [2, 1600000] int, W: [128,128], b,
prelu_a: [128].

Strategy (aggregation commutes with the linear map): out = PReLU((A_hat@x)@W+b).
Nodes are split into 8 contiguous ranges of 12500 (one per core). Edges
(+self loops) are partitioned by dst core and sorted by dst, so scatter-add is
core-local. Each core keeps the full x table in its DRAM and:
  - gathers x[src] rows with indirect DMA (the dominant, memory-bound step)
  - builds H[e, j] = norm_e * (dstloc_e == j) in one fused DVE tensor_scalar
  - PE accumulates accT[ch, node] += rows.T @ H per 128-node window in PSUM
  - per window: z = accT.T @ W + b (PE, rank-1 trick for b), PReLU (DVE), DMA
No collectives. Host work is index/sharding prep only (sort, degree counts,
per-edge norm scalars, padding to 128-edge tiles uniform across cores).

Implementation notes for this toolchain:
  - the program must be built as bacc.Bacc and .compile()d so bacc's
    generate_event_semaphores pass splits multi-sem waits (walrus codegen
    accepts at most ~2 sync commands per instruction).
  - HW indirect DMA honors ONE dynamic offset per partition (extra offset
    columns are ignored; it streams consecutive rows), so each gather moves
    exactly 128 rows (k_gather = 1). CoreSim models multi-offset gathers,
    hardware does not.
"""

import math

import numpy as np

import concourse.bacc as bacc
import concourse.mybir as mybir
import concourse.tile as tile
from concourse.bass import IndirectOffsetOnAxis
from concourse.bass_utils import run_bass_kernel_spmd

P = 128
N_CORES = 8
N_NODES = 100000
K_GATHER = 1

F32 = mybir.dt.float32
I32 = mybir.dt.int32


def _build_program(n_table_rows, n_out_rows, win_tiles, k_gather=K_GATHER):
    n_win = len(win_tiles)
    assert n_win == math.ceil(n_out_rows / P)
    T = int(sum(win_tiles))  # total 128-edge tiles

    tile_win = np.repeat(np.arange(n_win), win_tiles)
    win_start = np.zeros(n_win, dtype=np.int64)
    np.cumsum(win_tiles[:-1], out=win_start[1:])

    nc = bacc.Bacc("TRN2", target_bir_lowering=False)
    x = nc.declare_dram_parameter("x", [n_table_rows, P], F32, isOutput=False)
    # meta columns: [0:T) dstloc f32, [T:2T) norm f32, [2T:3T) src int32 bits
    meta = nc.declare_dram_parameter("meta", [P, 3 * T], F32, isOutput=False)
    w_p = nc.declare_dram_parameter("W", [P, P], F32, isOutput=False)
    b_p = nc.declare_dram_parameter("b", [1, P], F32, isOutput=False)
    a_p = nc.declare_dram_parameter("prelu_bcast", [P, P], F32, isOutput=False)
    iota_p = nc.declare_dram_parameter("iota", [P, P], F32, isOutput=False)
    y = nc.declare_dram_parameter("y", [n_out_rows, P], F32, isOutput=True)

    with tile.TileContext(nc) as tc:
        with (
            tc.tile_pool(name="meta", bufs=1) as meta_pool,
            tc.tile_pool(name="const", bufs=1) as const_pool,
            tc.tile_pool(name="rows", bufs=8) as rows_pool,
            tc.tile_pool(name="h", bufs=8) as h_pool,
            tc.tile_pool(name="epi", bufs=3) as epi_pool,
            tc.tile_pool(name="psum", bufs=2, space="PSUM") as psum_pool,
        ):
            meta_t = meta_pool.tile([P, 3 * T], F32, tag="meta")
            nc.sync.dma_start(out=meta_t[:], in_=meta[:, :])

            w_t = const_pool.tile([P, P], F32, tag="W")
            a_t = const_pool.tile([P, P], F32, tag="prelu")
            iota_t = const_pool.tile([P, P], F32, tag="iota")
            b_t = const_pool.tile([1, P], F32, tag="b")
            ones_t = const_pool.tile([1, P], F32, tag="ones")
            nc.sync.dma_start(out=w_t[:], in_=w_p[:, :])
            nc.sync.dma_start(out=a_t[:], in_=a_p[:, :])
            nc.sync.dma_start(out=iota_t[:], in_=iota_p[:, :])
            nc.sync.dma_start(out=b_t[:], in_=b_p[:, :])
            nc.vector.memset(ones_t[:], 1.0)

            accT = None

            def epilogue(w, accT_tile):
                r0 = w * P
                nr = min(P, n_out_rows - r0)
                accT_sb = epi_pool.tile([P, P], F32, tag="accT_sb")
                nc.vector.tensor_copy(out=accT_sb[:], in_=accT_tile[:])
                outp = psum_pool.tile([P, P], F32, tag="outp")
                nc.tensor.matmul(
                    out=outp[:], lhsT=accT_sb[:], rhs=w_t[:], start=True, stop=False
                )
                nc.tensor.matmul(
                    out=outp[:], lhsT=ones_t[:], rhs=b_t[:], start=False, stop=True
                )
                zpos = epi_pool.tile([P, P], F32, tag="zpos")
                nc.vector.tensor_scalar(
                    out=zpos[:],
                    in0=outp[:],
                    scalar1=0.0,
                    scalar2=None,
                    op0=mybir.AluOpType.max,
                )
                zneg = epi_pool.tile([P, P], F32, tag="zneg")
                nc.vector.tensor_tensor(
                    out=zneg[:], in0=outp[:], in1=zpos[:], op=mybir.AluOpType.subtract
                )
                zs = epi_pool.tile([P, P], F32, tag="zs")
                nc.vector.tensor_tensor(
                    out=zs[:], in0=zneg[:], in1=a_t[:], op=mybir.AluOpType.mult
                )
                out_sb = epi_pool.tile([P, P], F32, tag="out_sb")
                nc.vector.tensor_tensor(
                    out=out_sb[:], in0=zpos[:], in1=zs[:], op=mybir.AluOpType.add
                )
                nc.sync.dma_start(out=y[r0 : r0 + nr, :], in_=out_sb[:nr, :])

            for c0 in range(0, T, k_gather):
                k = min(k_gather, T - c0)
                rows = rows_pool.tile([P, k_gather * P], F32, tag="rows")
                nc.gpsimd.indirect_dma_start(
                    out=rows[:, : k * P],
                    out_offset=None,
                    in_=x[:, :],
                    in_offset=IndirectOffsetOnAxis(
                        ap=meta_t[:, 2 * T + c0 : 2 * T + c0 + k].bitcast(I32),
                        axis=0,
                    ),
                )
                for j in range(k):
                    t = c0 + j
                    w = int(tile_win[t])
                    first = t == int(win_start[w])
                    last = t == int(win_start[w]) + int(win_tiles[w]) - 1
                    if first:
                        accT = psum_pool.tile([P, P], F32, tag="accT")
                    h_t = h_pool.tile([P, P], F32, tag="h")
                    nc.vector.tensor_scalar(
                        out=h_t[:],
                        in0=iota_t[:],
                        scalar1=meta_t[:, t : t + 1],
                        scalar2=meta_t[:, T + t : T + t + 1],
                        op0=mybir.AluOpType.is_equal,
                        op1=mybir.AluOpType.mult,
                    )
                    nc.tensor.matmul(
                        out=accT[:],
                        lhsT=rows[:, j * P : (j + 1) * P],
                        rhs=h_t[:],
                        start=first,
                        stop=last,
                    )
                    if last:
                        epilogue(w, accT)
    nc.compile()
    return nc


def _preprocess(x, edge_index, n_cores=N_CORES):
    N = x.shape[0]
    src = np.asarray(edge_index[0], dtype=np.int64)
    dst = np.asarray(edge_index[1], dtype=np.int64)
    loop = np.arange(N, dtype=np.int64)
    src = np.concatenate([src, loop])
    dst = np.concatenate([dst, loop])
    deg = np.bincount(dst, minlength=N)
    dinv = (1.0 / np.sqrt(deg.astype(np.float64))).astype(np.float32)
    norm = dinv[src] * dinv[dst]

    rows_per_core = N // n_cores
    n_win = math.ceil(rows_per_core / P)

    order = np.argsort(dst, kind="stable")
    src_s = src[order].astype(np.int32)
    dst_s = dst[order]
    norm_s = norm[order]

    core_id = dst_s // rows_per_core
    local = dst_s - core_id * rows_per_core
    win = local // P
    dstloc = (local % P).astype(np.float32)

    group = core_id * n_win + win  # non-decreasing (edges sorted by dst)
    counts = np.bincount(group, minlength=n_cores * n_win).reshape(n_cores, n_win)
    win_tiles = np.maximum(1, -(-counts.max(axis=0) // P))
    T = int(win_tiles.sum())
    win_tile_start = np.zeros(n_win, dtype=np.int64)
    np.cumsum(win_tiles[:-1], out=win_tile_start[1:])

    group_start = np.zeros(n_cores * n_win, dtype=np.int64)
    np.cumsum(counts.ravel()[:-1], out=group_start[1:])
    rank = np.arange(len(dst_s)) - group_start[group]
    slot = win_tile_start[win] * P + rank

    metas = []
    for c in range(n_cores):
        m = core_id == c
        dstloc_pad = np.zeros(T * P, dtype=np.float32)
        norm_pad = np.zeros(T * P, dtype=np.float32)
        src_pad = np.zeros(T * P, dtype=np.int32)
        s = slot[m]
        dstloc_pad[s] = dstloc[m]
        norm_pad[s] = norm_s[m]
        src_pad[s] = src_s[m]
        # [P, 3T]: tile t lives in column t; SBUF partition p = edge t*128+p
        meta = np.empty((P, 3 * T), dtype=np.float32)
        meta[:, 0:T] = dstloc_pad.reshape(T, P).T
        meta[:, T : 2 * T] = norm_pad.reshape(T, P).T
        meta[:, 2 * T : 3 * T] = src_pad.reshape(T, P).T.view(np.float32)
        metas.append({"meta": np.ascontiguousarray(meta)})
    return metas, [int(t) for t in win_tiles], rows_per_core


def _make_in_maps(x, W, b, prelu_a, metas):
    consts = {
        "x": np.ascontiguousarray(np.asarray(x, dtype=np.float32)),
        "W": np.ascontiguousarray(np.asarray(W, dtype=np.float32)),
        "b": np.asarray(b, dtype=np.float32).reshape(1, P),
        "prelu_bcast": np.ascontiguousarray(
            np.tile(np.asarray(prelu_a, dtype=np.float32), (P, 1))
        ),
        "iota": np.tile(np.arange(P, dtype=np.float32), (P, 1)),
    }
    return [{**consts, **metas[c]} for c in range(N_CORES)]


def build_all(x, edge_index, W, b, prelu_a):
    """Preprocess + build. Returns (nc, in_maps, rows_per_core)."""
    metas, win_tiles, rows_per_core = _preprocess(x, edge_index)
    nc = _build_program(
        n_table_rows=x.shape[0], n_out_rows=rows_per_core, win_tiles=win_tiles
    )
    return nc, _make_in_maps(x, W, b, prelu_a, metas), rows_per_core


def kernel(x, edge_index, W, b, prelu_a):
    nc, in_maps, _ = build_all(x, edge_index, W, b, prelu_a)
    res = run_bass_kernel_spmd(nc, in_maps, core_ids=list(range(N_CORES)))
    return np.concatenate([res.results[c]["y"] for c in range(N_CORES)], axis=0)


# revision 8
# speedup vs baseline: 1.3845x; 1.3845x over previous
"""GCN layer (GCNConv + PReLU) on TRN2, SPMD across 8 NeuronCores.

Problem: out = PReLU(A_hat @ (x @ W) + b), A_hat = D^-1/2 (A+I) D^-1/2,
x: [100000, 128] f32, edge_index: [2, 1600000] int, W: [128,128], b,
prelu_a: [128].

Strategy (aggregation commutes with the linear map): out = PReLU((A_hat@x)@W+b).
Nodes are split into 8 contiguous ranges of 12500 (one per core). Edges
(+self loops) are partitioned by dst core and sorted by dst, so scatter-add is
core-local. Each core keeps the full x table in its DRAM and:
  - gathers x[src] rows with indirect DMA (the dominant, memory-bound step)
  - builds H[e, j] = norm_e * (dstloc_e == j) in one fused DVE tensor_scalar
  - PE accumulates accT[ch, node] += rows.T @ H per 128-node window in PSUM
  - per window: z = accT.T @ W + b (PE, rank-1 trick for b), PReLU (DVE), DMA
No collectives. Host work is index/sharding prep only (sort, degree counts,
per-edge norm scalars, padding to 128-edge tiles uniform across cores).

Implementation notes for this toolchain:
  - the program must be built as bacc.Bacc and .compile()d so bacc's
    generate_event_semaphores pass splits multi-sem waits (walrus codegen
    accepts at most ~2 sync commands per instruction).
  - HW indirect DMA honors ONE dynamic offset per partition (extra offset
    columns are ignored; it streams consecutive rows), so each gather moves
    exactly 128 rows (k_gather = 1). CoreSim models multi-offset gathers,
    hardware does not.
"""

import math

import numpy as np

import concourse.bacc as bacc
import concourse.mybir as mybir
import concourse.tile as tile
from concourse.bass import IndirectOffsetOnAxis
from concourse.bass_utils import run_bass_kernel_spmd

P = 128
N_CORES = 8
N_NODES = 100000
K_GATHER = 1

F32 = mybir.dt.float32
I32 = mybir.dt.int32


def _build_program(n_table_rows, n_out_rows, win_tiles, k_gather=K_GATHER):
    n_win = len(win_tiles)
    assert n_win == math.ceil(n_out_rows / P)
    T = int(sum(win_tiles))  # total 128-edge tiles

    tile_win = np.repeat(np.arange(n_win), win_tiles)
    win_start = np.zeros(n_win, dtype=np.int64)
    np.cumsum(win_tiles[:-1], out=win_start[1:])

    nc = bacc.Bacc("TRN2", target_bir_lowering=False)
    x = nc.declare_dram_parameter("x", [n_table_rows, P], F32, isOutput=False)
    # meta columns: [0:T) dstloc f32, [T:2T) norm f32, [2T:3T) src int32 bits
    meta = nc.declare_dram_parameter("meta", [P, 3 * T], F32, isOutput=False)
    w_p = nc.declare_dram_parameter("W", [P, P], F32, isOutput=False)
    b_p = nc.declare_dram_parameter("b", [1, P], F32, isOutput=False)
    a_p = nc.declare_dram_parameter("prelu_bcast", [P, P], F32, isOutput=False)
    iota_p = nc.declare_dram_parameter("iota", [P, P], F32, isOutput=False)
    y = nc.declare_dram_parameter("y", [n_out_rows, P], F32, isOutput=True)

    with tile.TileContext(nc) as tc:
        with (
            tc.tile_pool(name="meta", bufs=1) as meta_pool,
            tc.tile_pool(name="const", bufs=1) as const_pool,
            tc.tile_pool(name="rows", bufs=24) as rows_pool,
            tc.tile_pool(name="h", bufs=16) as h_pool,
            tc.tile_pool(name="epi", bufs=3) as epi_pool,
            tc.tile_pool(name="psum", bufs=2, space="PSUM") as psum_pool,
        ):
            meta_t = meta_pool.tile([P, 3 * T], F32, tag="meta")
            nc.sync.dma_start(out=meta_t[:], in_=meta[:, :])

            w_t = const_pool.tile([P, P], F32, tag="W")
            a_t = const_pool.tile([P, P], F32, tag="prelu")
            iota_t = const_pool.tile([P, P], F32, tag="iota")
            b_t = const_pool.tile([1, P], F32, tag="b")
            ones_t = const_pool.tile([1, P], F32, tag="ones")
            nc.sync.dma_start(out=w_t[:], in_=w_p[:, :])
            nc.sync.dma_start(out=a_t[:], in_=a_p[:, :])
            nc.sync.dma_start(out=iota_t[:], in_=iota_p[:, :])
            nc.sync.dma_start(out=b_t[:], in_=b_p[:, :])
            nc.vector.memset(ones_t[:], 1.0)

            accT = None

            def epilogue(w, accT_tile):
                r0 = w * P
                nr = min(P, n_out_rows - r0)
                accT_sb = epi_pool.tile([P, P], F32, tag="accT_sb")
                nc.vector.tensor_copy(out=accT_sb[:], in_=accT_tile[:])
                outp = psum_pool.tile([P, P], F32, tag="outp")
                nc.tensor.matmul(
                    out=outp[:], lhsT=accT_sb[:], rhs=w_t[:], start=True, stop=False
                )
                nc.tensor.matmul(
                    out=outp[:], lhsT=ones_t[:], rhs=b_t[:], start=False, stop=True
                )
                zpos = epi_pool.tile([P, P], F32, tag="zpos")
                nc.vector.tensor_scalar(
                    out=zpos[:],
                    in0=outp[:],
                    scalar1=0.0,
                    scalar2=None,
                    op0=mybir.AluOpType.max,
                )
                zneg = epi_pool.tile([P, P], F32, tag="zneg")
                nc.vector.tensor_tensor(
                    out=zneg[:], in0=outp[:], in1=zpos[:], op=mybir.AluOpType.subtract
                )
                zs = epi_pool.tile([P, P], F32, tag="zs")
                nc.vector.tensor_tensor(
                    out=zs[:], in0=zneg[:], in1=a_t[:], op=mybir.AluOpType.mult
                )
                out_sb = epi_pool.tile([P, P], F32, tag="out_sb")
                nc.vector.tensor_tensor(
                    out=out_sb[:], in0=zpos[:], in1=zs[:], op=mybir.AluOpType.add
                )
                nc.sync.dma_start(out=y[r0 : r0 + nr, :], in_=out_sb[:nr, :])

            for c0 in range(0, T, k_gather):
                k = min(k_gather, T - c0)
                rows = rows_pool.tile([P, k_gather * P], F32, tag="rows")
                nc.gpsimd.indirect_dma_start(
                    out=rows[:, : k * P],
                    out_offset=None,
                    in_=x[:, :],
                    in_offset=IndirectOffsetOnAxis(
                        ap=meta_t[:, 2 * T + c0 : 2 * T + c0 + k].bitcast(I32),
                        axis=0,
                    ),
                )
                for j in range(k):
                    t = c0 + j
                    w = int(tile_win[t])
                    first = t == int(win_start[w])
                    last = t == int(win_start[w]) + int(win_tiles[w]) - 1
                    if first:
                        accT = psum_pool.tile([P, P], F32, tag="accT")
                    h_t = h_pool.tile([P, P], F32, tag="h")
                    nc.vector.tensor_scalar(
                        out=h_t[:],
                        in0=iota_t[:],
                        scalar1=meta_t[:, t : t + 1],
                        scalar2=meta_t[:, T + t : T + t + 1],
                        op0=mybir.AluOpType.is_equal,
                        op1=mybir.AluOpType.mult,
                    )
                    nc.tensor.matmul(
                        out=accT[:],
                        lhsT=rows[:, j * P : (j + 1) * P],
                        rhs=h_t[:],
                        start=first,
                        stop=last,
                    )
                    if last:
                        epilogue(w, accT)
    nc.compile()
    return nc


def _preprocess(x, edge_index, n_cores=N_CORES):
    N = x.shape[0]
    src = np.asarray(edge_index[0], dtype=np.int64)
    dst = np.asarray(edge_index[1], dtype=np.int64)
    loop = np.arange(N, dtype=np.int64)
    src = np.concatenate([src, loop])
    dst = np.concatenate([dst, loop])
    deg = np.bincount(dst, minlength=N)
    dinv = (1.0 / np.sqrt(deg.astype(np.float64))).astype(np.float32)
    norm = dinv[src] * dinv[dst]

    rows_per_core = N // n_cores
    n_win = math.ceil(rows_per_core / P)

    order = np.argsort(dst, kind="stable")
    src_s = src[order].astype(np.int32)
    dst_s = dst[order]
    norm_s = norm[order]

    core_id = dst_s // rows_per_core
    local = dst_s - core_id * rows_per_core
    win = local // P
    dstloc = (local % P).astype(np.float32)

    group = core_id * n_win + win  # non-decreasing (edges sorted by dst)
    counts = np.bincount(group, minlength=n_cores * n_win).reshape(n_cores, n_win)
    win_tiles = np.maximum(1, -(-counts.max(axis=0) // P))
    T = int(win_tiles.sum())
    win_tile_start = np.zeros(n_win, dtype=np.int64)
    np.cumsum(win_tiles[:-1], out=win_tile_start[1:])

    group_start = np.zeros(n_cores * n_win, dtype=np.int64)
    np.cumsum(counts.ravel()[:-1], out=group_start[1:])
    rank = np.arange(len(dst_s)) - group_start[group]
    slot = win_tile_start[win] * P + rank

    metas = []
    for c in range(n_cores):
        m = core_id == c
        dstloc_pad = np.zeros(T * P, dtype=np.float32)
        norm_pad = np.zeros(T * P, dtype=np.float32)
        src_pad = np.zeros(T * P, dtype=np.int32)
        s = slot[m]
        dstloc_pad[s] = dstloc[m]
        norm_pad[s] = norm_s[m]
        src_pad[s] = src_s[m]
        # [P, 3T]: tile t lives in column t; SBUF partition p = edge t*128+p
        meta = np.empty((P, 3 * T), dtype=np.float32)
        meta[:, 0:T] = dstloc_pad.reshape(T, P).T
        meta[:, T : 2 * T] = norm_pad.reshape(T, P).T
        meta[:, 2 * T : 3 * T] = src_pad.reshape(T, P).T.view(np.float32)
        metas.append({"meta": np.ascontiguousarray(meta)})
    return metas, [int(t) for t in win_tiles], rows_per_core


def _make_in_maps(x, W, b, prelu_a, metas):
    consts = {
        "x": np.ascontiguousarray(np.asarray(x, dtype=np.float32)),
        "W": np.ascontiguousarray(np.asarray(W, dtype=np.float32)),
        "b": np.asarray(b, dtype=np.float32).reshape(1, P),
        "prelu_bcast": np.ascontiguousarray(
            np.tile(np.asarray(prelu_a, dtype=np.float32), (P, 1))
        ),
        "iota": np.tile(np.arange(P, dtype=np.float32), (P, 1)),
    }
    return [{**consts, **metas[c]} for c in range(N_CORES)]


def build_all(x, edge_index, W, b, prelu_a):
    """Preprocess + build. Returns (nc, in_maps, rows_per_core)."""
    metas, win_tiles, rows_per_core = _preprocess(x, edge_index)
    nc = _build_program(
        n_table_rows=x.shape[0], n_out_rows=rows_per_core, win_tiles=win_tiles
    )
    return nc, _make_in_maps(x, W, b, prelu_a, metas), rows_per_core


def kernel(x, edge_index, W, b, prelu_a):
    nc, in_maps, _ = build_all(x, edge_index, W, b, prelu_a)
    res = run_bass_kernel_spmd(nc, in_maps, core_ids=list(range(N_CORES)))
    return np.concatenate([res.results[c]["y"] for c in range(N_CORES)], axis=0)


# revision 9
# speedup vs baseline: 2.2828x; 1.6489x over previous
"""GCN layer (GCNConv + PReLU) on TRN2, SPMD across 8 NeuronCores.

Problem: out = PReLU(A_hat @ (x @ W) + b), A_hat = D^-1/2 (A+I) D^-1/2,
x: [100000, 128] f32, edge_index: [2, 1600000] int, W: [128,128], b,
prelu_a: [128].

Strategy (aggregation commutes with the linear map): out = PReLU((A_hat@x)@W+b).
Nodes are split into 8 contiguous ranges of 12500 (one per core). Edges
(+self loops) are partitioned by dst core and sorted by dst, so scatter-add is
core-local. Each core keeps the full x table in its DRAM and:
  - gathers x[src] rows with indirect DMA (the dominant, memory-bound step)
  - builds H[e, j] = norm_e * (dstloc_e == j) in one fused DVE tensor_scalar
  - PE accumulates accT[ch, node] += rows.T @ H per 128-node window in PSUM
  - per window: z = accT.T @ W + b (PE, rank-1 trick for b), PReLU (DVE), DMA
No collectives. Host work is index/sharding prep only (sort, degree counts,
per-edge norm scalars, padding to 128-edge tiles uniform across cores).

Implementation notes for this toolchain:
  - the program must be built as bacc.Bacc and .compile()d so bacc's
    generate_event_semaphores pass splits multi-sem waits (walrus codegen
    accepts at most ~2 sync commands per instruction).
  - HW indirect DMA honors ONE dynamic offset per partition (extra offset
    columns are ignored; it streams consecutive rows), so each gather moves
    exactly 128 rows (k_gather = 1). CoreSim models multi-offset gathers,
    hardware does not.
"""

import math

import numpy as np

import concourse.bacc as bacc
import concourse.mybir as mybir
import concourse.tile as tile
from concourse.bass import IndirectOffsetOnAxis
from concourse.bass_utils import run_bass_kernel_spmd

P = 128
N_CORES = 8
N_NODES = 100000
K_GATHER = 1

F32 = mybir.dt.float32
I32 = mybir.dt.int32


def _build_program(n_table_rows, n_out_rows, win_tiles, k_gather=K_GATHER):
    n_win = len(win_tiles)
    assert n_win == math.ceil(n_out_rows / P)
    T = int(sum(win_tiles))  # total 128-edge tiles

    tile_win = np.repeat(np.arange(n_win), win_tiles)
    win_start = np.zeros(n_win, dtype=np.int64)
    np.cumsum(win_tiles[:-1], out=win_start[1:])

    nc = bacc.Bacc("TRN2", target_bir_lowering=False)
    x = nc.declare_dram_parameter("x", [n_table_rows, P], F32, isOutput=False)
    # meta columns: [0:T) dstloc f32, [T:2T) norm f32, [2T:3T) src int32 bits
    meta = nc.declare_dram_parameter("meta", [P, 3 * T], F32, isOutput=False)
    w_p = nc.declare_dram_parameter("W", [P, P], F32, isOutput=False)
    b_p = nc.declare_dram_parameter("b", [1, P], F32, isOutput=False)
    a_p = nc.declare_dram_parameter("prelu_bcast", [P, P], F32, isOutput=False)
    iota_p = nc.declare_dram_parameter("iota", [P, P], F32, isOutput=False)
    y = nc.declare_dram_parameter("y", [n_out_rows, P], F32, isOutput=True)

    with tile.TileContext(nc) as tc:
        with (
            tc.tile_pool(name="meta", bufs=1) as meta_pool,
            tc.tile_pool(name="const", bufs=1) as const_pool,
            tc.tile_pool(name="rows", bufs=24) as rows_pool,
            tc.tile_pool(name="h", bufs=16) as h_pool,
            tc.tile_pool(name="epi", bufs=3) as epi_pool,
            tc.tile_pool(name="psum", bufs=2, space="PSUM") as psum_pool,
        ):
            meta_t = meta_pool.tile([P, 3 * T], F32, tag="meta")
            nc.sync.dma_start(out=meta_t[:], in_=meta[:, :])

            w_t = const_pool.tile([P, P], F32, tag="W")
            a_t = const_pool.tile([P, P], F32, tag="prelu")
            iota_t = const_pool.tile([P, P], F32, tag="iota")
            b_t = const_pool.tile([1, P], F32, tag="b")
            ones_t = const_pool.tile([1, P], F32, tag="ones")
            nc.sync.dma_start(out=w_t[:], in_=w_p[:, :])
            nc.sync.dma_start(out=a_t[:], in_=a_p[:, :])
            nc.sync.dma_start(out=iota_t[:], in_=iota_p[:, :])
            nc.sync.dma_start(out=b_t[:], in_=b_p[:, :])
            nc.vector.memset(ones_t[:], 1.0)

            accT = None

            def epilogue(w, accT_tile):
                r0 = w * P
                nr = min(P, n_out_rows - r0)
                accT_sb = epi_pool.tile([P, P], F32, tag="accT_sb")
                nc.vector.tensor_copy(out=accT_sb[:], in_=accT_tile[:])
                outp = psum_pool.tile([P, P], F32, tag="outp")
                nc.tensor.matmul(
                    out=outp[:], lhsT=accT_sb[:], rhs=w_t[:], start=True, stop=False
                )
                nc.tensor.matmul(
                    out=outp[:], lhsT=ones_t[:], rhs=b_t[:], start=False, stop=True
                )
                zpos = epi_pool.tile([P, P], F32, tag="zpos")
                nc.vector.tensor_scalar(
                    out=zpos[:],
                    in0=outp[:],
                    scalar1=0.0,
                    scalar2=None,
                    op0=mybir.AluOpType.max,
                )
                zneg = epi_pool.tile([P, P], F32, tag="zneg")
                nc.vector.tensor_tensor(
                    out=zneg[:], in0=outp[:], in1=zpos[:], op=mybir.AluOpType.subtract
                )
                zs = epi_pool.tile([P, P], F32, tag="zs")
                nc.vector.tensor_tensor(
                    out=zs[:], in0=zneg[:], in1=a_t[:], op=mybir.AluOpType.mult
                )
                out_sb = epi_pool.tile([P, P], F32, tag="out_sb")
                nc.vector.tensor_tensor(
                    out=out_sb[:], in0=zpos[:], in1=zs[:], op=mybir.AluOpType.add
                )
                nc.sync.dma_start(out=y[r0 : r0 + nr, :], in_=out_sb[:nr, :])

            for c0 in range(0, T, k_gather):
                k = min(k_gather, T - c0)
                rows = rows_pool.tile([P, k_gather * P], F32, tag="rows")
                nc.gpsimd.indirect_dma_start(
                    out=rows[:, : k * P],
                    out_offset=None,
                    in_=x[:, :],
                    in_offset=IndirectOffsetOnAxis(
                        ap=meta_t[:, 2 * T + c0 : 2 * T + c0 + k].bitcast(I32),
                        axis=0,
                    ),
                )
                for j in range(k):
                    t = c0 + j
                    w = int(tile_win[t])
                    first = t == int(win_start[w])
                    last = t == int(win_start[w]) + int(win_tiles[w]) - 1
                    if first:
                        accT = psum_pool.tile([P, P], F32, tag="accT")
                    h_t = h_pool.tile([P, P], F32, tag="h")
                    nc.vector.tensor_scalar(
                        out=h_t[:],
                        in0=iota_t[:],
                        scalar1=meta_t[:, t : t + 1],
                        scalar2=meta_t[:, T + t : T + t + 1],
                        op0=mybir.AluOpType.is_equal,
                        op1=mybir.AluOpType.mult,
                    )
                    nc.tensor.matmul(
                        out=accT[:],
                        lhsT=rows[:, j * P : (j + 1) * P],
                        rhs=h_t[:],
                        start=first,
                        stop=last,
                    )
                    if last:
                        epilogue(w, accT)
    nc.compile()
    return nc


def _preprocess(x, edge_index, n_cores=N_CORES):
    N = x.shape[0]
    src = np.asarray(edge_index[0], dtype=np.int64)
    dst = np.asarray(edge_index[1], dtype=np.int64)
    loop = np.arange(N, dtype=np.int64)
    src = np.concatenate([src, loop])
    dst = np.concatenate([dst, loop])
    deg = np.bincount(dst, minlength=N)
    dinv = (1.0 / np.sqrt(deg.astype(np.float64))).astype(np.float32)
    norm = dinv[src] * dinv[dst]

    rows_per_core = N // n_cores
    n_win = math.ceil(rows_per_core / P)

    order = np.argsort(dst, kind="stable")
    src_s = src[order].astype(np.int32)
    dst_s = dst[order]
    norm_s = norm[order]

    core_id = dst_s // rows_per_core
    local = dst_s - core_id * rows_per_core
    win = local // P
    dstloc = (local % P).astype(np.float32)

    group = core_id * n_win + win  # non-decreasing (edges sorted by dst)
    counts = np.bincount(group, minlength=n_cores * n_win).reshape(n_cores, n_win)
    # Deal each core's windows to slots in count-sorted order so the SPMD
    # max-across-cores tile count per slot shrinks toward the per-core ideal.
    # The short last window (rows_per_core % 128) stays pinned at the last slot.
    perm = np.empty((n_cores, n_win), dtype=np.int64)  # perm[c, slot] = window
    for c in range(n_cores):
        perm[c, : n_win - 1] = np.argsort(-counts[c, : n_win - 1], kind="stable")
        perm[c, n_win - 1] = n_win - 1
    inv_perm = np.empty_like(perm)  # inv_perm[c, window] = slot
    np.put_along_axis(inv_perm, perm, np.arange(n_win)[None, :], axis=1)
    slot_counts = np.take_along_axis(counts, perm, axis=1)
    win_tiles = np.maximum(1, -(-slot_counts.max(axis=0) // P))
    T = int(win_tiles.sum())
    win_tile_start = np.zeros(n_win, dtype=np.int64)
    np.cumsum(win_tiles[:-1], out=win_tile_start[1:])

    group_start = np.zeros(n_cores * n_win, dtype=np.int64)
    np.cumsum(counts.ravel()[:-1], out=group_start[1:])
    rank = np.arange(len(dst_s)) - group_start[group]
    edge_slot = inv_perm[core_id, win]
    slot = win_tile_start[edge_slot] * P + rank

    metas = []
    for c in range(n_cores):
        m = core_id == c
        dstloc_pad = np.zeros(T * P, dtype=np.float32)
        norm_pad = np.zeros(T * P, dtype=np.float32)
        src_pad = np.zeros(T * P, dtype=np.int32)
        s = slot[m]
        dstloc_pad[s] = dstloc[m]
        norm_pad[s] = norm_s[m]
        src_pad[s] = src_s[m]
        # [P, 3T]: tile t lives in column t; SBUF partition p = edge t*128+p
        meta = np.empty((P, 3 * T), dtype=np.float32)
        meta[:, 0:T] = dstloc_pad.reshape(T, P).T
        meta[:, T : 2 * T] = norm_pad.reshape(T, P).T
        meta[:, 2 * T : 3 * T] = src_pad.reshape(T, P).T.view(np.float32)
        metas.append({"meta": np.ascontiguousarray(meta)})
    return metas, [int(t) for t in win_tiles], rows_per_core, perm


def _make_in_maps(x, W, b, prelu_a, metas):
    consts = {
        "x": np.ascontiguousarray(np.asarray(x, dtype=np.float32)),
        "W": np.ascontiguousarray(np.asarray(W, dtype=np.float32)),
        "b": np.asarray(b, dtype=np.float32).reshape(1, P),
        "prelu_bcast": np.ascontiguousarray(
            np.tile(np.asarray(prelu_a, dtype=np.float32), (P, 1))
        ),
        "iota": np.tile(np.arange(P, dtype=np.float32), (P, 1)),
    }
    return [{**consts, **metas[c]} for c in range(N_CORES)]


def _unscramble(y_slot_order, perm, rows_per_core):
    """y rows are in per-core slot order; map slot s -> window perm[c, s]."""
    n_win = perm.shape[1]
    out = np.empty_like(y_slot_order)
    for c in range(perm.shape[0]):
        yc = y_slot_order[c * rows_per_core : (c + 1) * rows_per_core]
        oc = out[c * rows_per_core : (c + 1) * rows_per_core]
        for s in range(n_win):
            w = int(perm[c, s])
            nr = min(P, rows_per_core - w * P)
            oc[w * P : w * P + nr] = yc[s * P : s * P + nr]
    return out


def build_all(x, edge_index, W, b, prelu_a):
    """Preprocess + build. Returns (nc, in_maps, rows_per_core, unscramble)."""
    metas, win_tiles, rows_per_core, perm = _preprocess(x, edge_index)
    nc = _build_program(
        n_table_rows=x.shape[0], n_out_rows=rows_per_core, win_tiles=win_tiles
    )
    unscramble = lambda y: _unscramble(y, perm, rows_per_core)
    return nc, _make_in_maps(x, W, b, prelu_a, metas), rows_per_core, unscramble


def kernel(x, edge_index, W, b, prelu_a):
    nc, in_maps, _, unscramble = build_all(x, edge_index, W, b, prelu_a)
    res = run_bass_kernel_spmd(nc, in_maps, core_ids=list(range(N_CORES)))
    y = np.concatenate([res.results[c]["y"] for c in range(N_CORES)], axis=0)
    return unscramble(y)
